# revision 15
# baseline (speedup 1.0000x reference)
"""Trainium2 Bass kernel for batched MultiHeadAttention + LayerNorm.

Computes, per batch element b (B=1024, S=50, D=512, H=8 heads, Dk=64):
    q = Q @ W_Q.T ; k = K @ W_K.T ; v = V @ W_V.T          (per-head split)
    scores = q k^T / sqrt(Dk)  (+ mask)   attn = softmax(scores) * time_weight
    context = attn @ v ;  out = LayerNorm(context @ W_fc.T + Q)

Sharding: data-parallel over batch across 8 NeuronCores (128 batches/core).
Weights replicated. All matmuls in bf16 (PE cannot LDWEIGHTS fp32); fp32
accumulation in PSUM; LayerNorm in fp32.

Device layout strategy (per core):
  - Q,K shipped feature-major (QT/KT [512, 6400] bf16) so the d-contraction
    projections need no on-device transpose.
  - V and residual-Q shipped feature-major/token-major with each batch padded
    to 64 rows (t = b*64 + s) so per-batch rows are 64-aligned for PE
    partition slicing.
  - scores computed transposed (scoresT[kt, qt] = k q^T): softmax sum over kt
    is done on the PE (expT @ ones), and expT is directly the stationary
    operand of the context matmul -> no attention transposes.
  - context comes out token-major [qt, e]; normalization by 1/rowsum is fused
    into the PSUM->SBUF evacuation multiply; DMA-xbar transposes (bf16) give
    the feature-major tiles for the final projection.
  - residual add is a PE matmul with the identity (accumulates into the same
    PSUM bank as the final projection); LayerNorm via bn_stats/bn_aggr +
    fused (x - mu) * rstd tensor_scalar.
"""

import sys

sys.path.insert(0, "/opt/trn_rl_repo")

import numpy as np
import ml_dtypes

import concourse.bass as bass
import concourse.tile as tile
from concourse import bacc, mybir
from concourse.bass_utils import run_bass_kernel_spmd
from concourse.masks import make_identity

BF16 = mybir.dt.bfloat16
FP32 = mybir.dt.float32

B = 1024
S = 50
D = 512
H = 8
DK = 64
NCORES = 8
BC = B // NCORES          # batches per core = 128
SPAD = 64                 # padded seq rows per batch
NB = 16                   # batches per chunk
NCHUNK = BC // NB         # 8 chunks
CT = NB * S               # real tokens per chunk = 800
CTP = NB * SPAD           # padded tokens per chunk = 1024
T = BC * S                # real tokens per core = 6400
TP = BC * SPAD            # padded tokens per core = 8192
NEG = -1e9
LN_EPS = 1e-5
INV_SQRT_DK = 1.0 / 8.0


def _build_nc(n_chunk=NCHUNK, use_tw=False, use_mask=False, stage=9):
    nc = bacc.Bacc("TRN2", target_bir_lowering=False, debug=False)

    t_tot = n_chunk * CT
    tp_tot = n_chunk * CTP
    qt_d = nc.dram_tensor("QT", [D, t_tot], BF16, kind="ExternalInput")
    kt_d = nc.dram_tensor("KT", [D, t_tot], BF16, kind="ExternalInput")
    vt_d = nc.dram_tensor("VT", [D, tp_tot], BF16, kind="ExternalInput")
    qres_d = nc.dram_tensor("QRES", [tp_tot, D], BF16, kind="ExternalInput")
    wqt_d = nc.dram_tensor("WQT", [D, D], BF16, kind="ExternalInput")
    wkt_d = nc.dram_tensor("WKT", [D, D], BF16, kind="ExternalInput")
    wvt_d = nc.dram_tensor("WVT", [D, D], BF16, kind="ExternalInput")
    wfct_d = nc.dram_tensor("WFCT", [D, D], BF16, kind="ExternalInput")
    out_d = nc.dram_tensor("OUT", [t_tot, D], FP32, kind="ExternalOutput")
    if use_tw:
        # twT[h] at rows 64*(h%2), cols 50*h: [128, H*S]
        twt_d = nc.dram_tensor("TWT", [2 * SPAD, H * S], BF16, kind="ExternalInput")
    if use_mask:
        # additive mask (pre-scaled by sqrt(dk)), scoresT layout per batch,
        # head h block at rows 64*(h%2), cols 50*h
        mask_d = nc.dram_tensor(
            "MASKNEG", [n_chunk * NB, 2 * SPAD, H * S], FP32,
            kind="ExternalInput",
        )

    from contextlib import ExitStack
    with tile.TileContext(nc) as tc, ExitStack() as ctx:
        consts = ctx.enter_context(tc.tile_pool(name="consts", bufs=1))
        sb = ctx.enter_context(tc.tile_pool(name="sb", bufs=1))
        ps = ctx.enter_context(tc.tile_pool(name="ps", bufs=1, space="PSUM"))

        # ---- persistent constants ----
        wq_sb = []
        wk_sb = []
        wv_sb = []
        wfc_sb = []
        for dt in range(4):
            t_q = consts.tile([128, D], BF16, name=f"wq{dt}", tag=f"wq{dt}")
            nc.sync.dma_start(out=t_q[:, :], in_=wqt_d[128 * dt:128 * (dt + 1), :])
            wq_sb.append(t_q)
            t_k = consts.tile([128, D], BF16, name=f"wk{dt}", tag=f"wk{dt}")
            nc.sync.dma_start(out=t_k[:, :], in_=wkt_d[128 * dt:128 * (dt + 1), :])
            wk_sb.append(t_k)
            t_v = consts.tile([128, D], BF16, name=f"wv{dt}", tag=f"wv{dt}")
            nc.sync.dma_start(out=t_v[:, :], in_=wvt_d[128 * dt:128 * (dt + 1), :])
            wv_sb.append(t_v)
            t_f = consts.tile([128, D], BF16, name=f"wfc{dt}", tag=f"wfc{dt}")
            nc.sync.dma_start(out=t_f[:, :], in_=wfct_d[128 * dt:128 * (dt + 1), :])
            wfc_sb.append(t_f)
        ident = consts.tile([128, 128], BF16, name="ident", tag="ident")
        make_identity(nc, ident[:, :])
        ones_sb = consts.tile([128, 1], BF16, name="ones", tag="ones")
        nc.vector.memset(ones_sb[:, :], 1.0)
        eps_sb = consts.tile([128, 1], FP32, name="eps", tag="eps")
        nc.vector.memset(eps_sb[:, :], LN_EPS)
        if use_tw:
            tw_sb = consts.tile([2 * SPAD, H * S], BF16, name="twsb", tag="twsb")
            nc.sync.dma_start(out=tw_sb[:, :], in_=twt_d[:, :])

        for ch in range(n_chunk):
            t0 = ch * CT          # real-token base of chunk
            tp0 = ch * CTP        # padded-token base of chunk

            # ---- load chunk inputs ----
            qtin = []
            ktin = []
            vtin = []
            for dt in range(4):
                ti_q = sb.tile([128, CT], BF16, name=f"qtin{dt}", tag="qtin", bufs=8)
                nc.sync.dma_start(
                    out=ti_q[:, :], in_=qt_d[128 * dt:128 * (dt + 1), t0:t0 + CT])
                qtin.append(ti_q)
                ti_k = sb.tile([128, CT], BF16, name=f"ktin{dt}", tag="ktin", bufs=8)
                nc.sync.dma_start(
                    out=ti_k[:, :], in_=kt_d[128 * dt:128 * (dt + 1), t0:t0 + CT])
                ktin.append(ti_k)
                ti_v = sb.tile([128, CTP], BF16, name=f"vtin{dt}", tag="vtin", bufs=8)
                nc.sync.dma_start(
                    out=ti_v[:, :], in_=vt_d[128 * dt:128 * (dt + 1), tp0:tp0 + CTP])
                vtin.append(ti_v)

            # ---- q/k projections -> feature-major qT/kT [4][128, CT] ----
            qT = []
            kT = []
            for et in range(4):
                tq = sb.tile([128, CT], BF16, name=f"qT{et}", tag="qT", bufs=8)
                tk = sb.tile([128, CT], BF16, name=f"kT{et}", tag="kT", bufs=8)
                for half in range(2):
                    hs = CT // 2
                    pq = ps.tile([128, hs], FP32, name="psq", tag="psA", bufs=3)
                    for dt in range(4):
                        nc.tensor.matmul(
                            pq[:, :],
                            lhsT=wq_sb[dt][:, 128 * et:128 * (et + 1)],
                            rhs=qtin[dt][:, half * hs:(half + 1) * hs],
                            start=(dt == 0), stop=(dt == 3),
                        )
                    nc.any.tensor_copy(tq[:, half * hs:(half + 1) * hs], pq[:, :])
                    pk = ps.tile([128, hs], FP32, name="psk", tag="psA", bufs=3)
                    for dt in range(4):
                        nc.tensor.matmul(
                            pk[:, :],
                            lhsT=wk_sb[dt][:, 128 * et:128 * (et + 1)],
                            rhs=ktin[dt][:, half * hs:(half + 1) * hs],
                            start=(dt == 0), stop=(dt == 3),
                        )
                    nc.any.tensor_copy(tk[:, half * hs:(half + 1) * hs], pk[:, :])
                qT.append(tq)
                kT.append(tk)

            # ---- v projection -> token-major padded v [NB//2][128, 512] ----
            v_c = []
            for tt in range(NB // 2):
                pv = ps.tile([128, D], FP32, name="psv", tag="psA", bufs=3)
                for dt in range(4):
                    nc.tensor.matmul(
                        pv[:, :],
                        lhsT=vtin[dt][:, 128 * tt:128 * (tt + 1)],
                        rhs=wv_sb[dt][:, :],
                        start=(dt == 0), stop=(dt == 3),
                    )
                tv = sb.tile([128, D], BF16, name=f"v{tt}", tag="vc", bufs=16)
                nc.any.tensor_copy(tv[:, :], pv[:, :])
                v_c.append(tv)

            # parity-swapped copy of v (batch rows moved to the other 64-block)
            v2_c = []
            for tt in range(NB // 2):
                tv2 = sb.tile([128, D], BF16, name=f"v2{tt}", tag="v2c", bufs=16)
                nc.sync.dma_start(out=tv2[0:64, :], in_=v_c[tt][64:128, :])
                nc.sync.dma_start(out=tv2[64:128, :], in_=v_c[tt][0:64, :])
                v2_c.append(tv2)

            if stage < 2:
                continue
            # ---- attention + output, per batch pair ----
            # All tile_positions are diagonal (r==c): partition blocks are
            # keyed by HEAD parity (head h lives in rows 64*(h%2)).
            for bp in range(NB // 2):
                c_pair = []
                for p in range(2):
                    b_loc = 2 * bp + p
                    tloc = S * b_loc
                    psc = ps.tile([128, 512], FP32, name="psc", tag="psB",
                                  bufs=2)
                    expT = sb.tile([128, 400], BF16, name="expT", tag="expT",
                                   bufs=4)
                    pc = ps.tile([128, D], FP32, name="pc", tag="psC", bufs=2)
                    nc.vector.memset(pc[:, :], 0.0)
                    r_sb = sb.tile([128, 4], FP32, name="rsb", tag="rsb",
                                   bufs=4)
                    nc.vector.memset(r_sb[:, :], 0.0)
                    if use_mask:
                        bi = ch * NB + b_loc
                        mrow = sb.tile([128, 400], FP32, name="mrow",
                                       tag="mrow", bufs=4)
                        nc.sync.dma_start(out=mrow[:, :], in_=mask_d[bi, :, :])
                    for h in range(8):
                        ph = h % 2
                        nc.tensor.matmul(
                            psc[64 * ph:64 * ph + S, S * h:S * (h + 1)],
                            lhsT=kT[h // 2][64 * ph:64 * ph + 64, tloc:tloc + S],
                            rhs=qT[h // 2][64 * ph:64 * ph + 64, tloc:tloc + S],
                            start=True, stop=True,
                            tile_position=(64 * ph, 64 * ph),
                        )
                    if stage < 3:
                        continue
                    for ph in range(2):
                        rows = slice(64 * ph, 64 * ph + S)
                        pv_sc = psc[rows, 0:400].rearrange(
                            "p (hh hp s) -> p hh hp s", hp=2, s=S)[:, :, ph, :]
                        ev_ex = expT[rows, :].rearrange(
                            "p (hh hp s) -> p hh hp s", hp=2, s=S)[:, :, ph, :]
                        if use_mask:
                            mv_ = mrow[rows, :].rearrange(
                                "p (hh hp s) -> p hh hp s",
                                hp=2, s=S)[:, :, ph, :]
                            nc.vector.tensor_add(pv_sc, pv_sc, mv_)
                        # exp(scores/sqrt(dk)); bf16 out is the context lhsT
                        nc.scalar.activation(
                            out=ev_ex, in_=pv_sc,
                            func=mybir.ActivationFunctionType.Exp,
                            scale=INV_SQRT_DK,
                        )
                        if use_tw:
                            tv_ = tw_sb[rows, :].rearrange(
                                "p (hh hp s) -> p hh hp s",
                                hp=2, s=S)[:, :, ph, :]
                            nc.vector.tensor_mul(ev_ex, ev_ex, tv_)
                    if stage < 4:
                        continue
                    for h in range(8):
                        ph = h % 2
                        rows = slice(64 * ph, 64 * ph + S)
                        lhsT = expT[rows, S * h:S * (h + 1)]
                        vsrc = v_c[bp] if ph == p else v2_c[bp]
                        nc.tensor.matmul(
                            pc[rows, DK * h:DK * (h + 1)],
                            lhsT=lhsT,
                            rhs=vsrc[rows, DK * h:DK * (h + 1)],
                            start=True, stop=True,
                            tile_position=(64 * ph, 64 * ph),
                        )
                        if stage >= 5:
                            nc.tensor.matmul(
                                psc[rows, 400 + h // 2:401 + h // 2],
                                lhsT=lhsT,
                                rhs=ones_sb[rows, :],
                                start=True, stop=True,
                                tile_position=(64 * ph, 64 * ph),
                            )
                    if stage < 5:
                        continue
                    nc.vector.reciprocal(r_sb[0:S, :], psc[0:S, 400:404])
                    nc.vector.reciprocal(r_sb[64:64 + S, :],
                                         psc[64:64 + S, 400:404])

                    if stage < 6:
                        continue
                    # normalize + evacuate context (bf16, head-parity rows)
                    c_sb = sb.tile([128, D], BF16, name="csb", tag="csb",
                                   bufs=4)
                    nc.vector.tensor_mul(
                        c_sb[:, :].rearrange("p (hh hp e) -> p hh hp e",
                                             hp=2, e=DK),
                        pc[:, :].rearrange("p (hh hp e) -> p hh hp e",
                                           hp=2, e=DK),
                        r_sb[:, :].to_broadcast([128, 4, 2, DK]),
                    )
                    c_pair.append(c_sb)

                if stage < 7:
                    continue
                # compact the pair into token-major rows (b0 -> 0:64,
                # b1 -> 64:128), merging head-parity blocks
                c2 = sb.tile([128, D], BF16, name="c2", tag="c2", bufs=4)
                for p in range(2):
                    cv = c_pair[p]
                    for hp in range(2):
                        src_view = cv[64 * hp:64 * hp + 64, :].rearrange(
                            "p (hh hp e) -> p hh hp e", hp=2, e=DK)[:, :, hp, :]
                        dst_view = c2[64 * p:64 * p + 64, :].rearrange(
                            "p (hh hp e) -> p hh hp e", hp=2, e=DK)[:, :, hp, :]
                        nc.sync.dma_start(out=dst_view, in_=src_view)

                # transpose context tiles to feature-major via DMA xbar
                cT = []
                for et in range(4):
                    tcte = sb.tile([128, 128], BF16, name=f"cT{et}",
                                   tag="cT", bufs=8)
                    nc.sync.dma_start(
                        out=tcte[:, :], in_=c2[:, 128 * et:128 * (et + 1)],
                        transpose=True)
                    cT.append(tcte)

                if stage < 8:
                    continue
                # residual tile
                qres_sb = sb.tile([128, D], BF16, name="qres", tag="qres",
                                  bufs=3)
                nc.sync.dma_start(
                    out=qres_sb[:, :],
                    in_=qres_d[tp0 + 128 * bp:tp0 + 128 * (bp + 1), :])

                # final projection + residual (PSUM accumulate)
                po = ps.tile([128, D], FP32, name="po", tag="psD", bufs=1)
                for et in range(4):
                    nc.tensor.matmul(
                        po[:, :], lhsT=cT[et][:, :], rhs=wfc_sb[et][:, :],
                        start=(et == 0), stop=False,
                    )
                nc.tensor.matmul(
                    po[:, :], lhsT=ident[:, :], rhs=qres_sb[:, :],
                    start=False, stop=True,
                )

                if stage < 9:
                    continue
                # LayerNorm over d
                stats = sb.tile([128, 6], FP32, name="stats", tag="stats",
                                bufs=4)
                mv = sb.tile([128, 2], FP32, name="mv", tag="mv", bufs=4)
                std = sb.tile([128, 1], FP32, name="std", tag="std", bufs=4)
                rstd = sb.tile([128, 1], FP32, name="rstd", tag="rstd", bufs=4)
                nc.vector.bn_stats(out=stats[:, :], in_=po[:, :])
                nc.vector.bn_aggr(out=mv[:, :], in_=stats[:, :])
                nc.scalar.activation(
                    out=std[:, :], in_=mv[:, 1:2],
                    func=mybir.ActivationFunctionType.Sqrt,
                    bias=eps_sb[:, :])
                nc.vector.reciprocal(rstd[:, :], std[:, :])
                out_sb = sb.tile([128, D], FP32, name="outsb", tag="outsb",
                                 bufs=3)
                nc.vector.tensor_scalar(
                    out=out_sb[:, :], in0=po[:, :],
                    scalar1=mv[:, 0:1], scalar2=rstd[:, :],
                    op0=mybir.AluOpType.subtract, op1=mybir.AluOpType.mult,
                )

                # store real rows
                b0 = ch * NB + 2 * bp
                nc.sync.dma_start(
                    out=out_d[S * b0:S * (b0 + 1), :], in_=out_sb[0:S, :])
                nc.sync.dma_start(
                    out=out_d[S * (b0 + 1):S * (b0 + 2), :],
                    in_=out_sb[64:64 + S, :])

    nc.compile()
    return nc


_NC_CACHE = {}


def _get_nc(n_chunk, use_tw, use_mask):
    key = (n_chunk, use_tw, use_mask)
    if key not in _NC_CACHE:
        _NC_CACHE[key] = _build_nc(n_chunk, use_tw, use_mask)
    return _NC_CACHE[key]


def _prep_core_inputs(Qc, Kc, Vc, n_chunk, weights, tw_extra, mask_extra):
    """Host-side layout prep for one core's shard (nb batches)."""
    nb = Qc.shape[0]
    t = nb * S
    tp = nb * SPAD
    bf = ml_dtypes.bfloat16
    qt = np.ascontiguousarray(Qc.reshape(t, D).T.astype(bf))
    kt = np.ascontiguousarray(Kc.reshape(t, D).T.astype(bf))
    vp = np.zeros((nb, SPAD, D), dtype=bf)
    vp[:, :S] = Vc
    vt = np.ascontiguousarray(vp.reshape(tp, D).T)
    qres = np.zeros((nb, SPAD, D), dtype=bf)
    qres[:, :S] = Qc
    qres = qres.reshape(tp, D)
    m = {"QT": qt, "KT": kt, "VT": vt, "QRES": qres}
    m.update(weights)
    m.update(tw_extra)
    m.update(mask_extra)
    return m


def kernel(Q, K, V, attn_mask, W_Q, W_K, W_V, W_fc, time_weight):
    Q = np.asarray(Q, dtype=np.float32)
    K = np.asarray(K, dtype=np.float32)
    V = np.asarray(V, dtype=np.float32)
    W_Q = np.asarray(W_Q, dtype=np.float32)
    W_K = np.asarray(W_K, dtype=np.float32)
    W_V = np.asarray(W_V, dtype=np.float32)
    W_fc = np.asarray(W_fc, dtype=np.float32)
    time_weight = np.asarray(time_weight, dtype=np.float32)
    attn_mask = np.asarray(attn_mask)

    bf = ml_dtypes.bfloat16
    use_tw = not np.all(time_weight == 1.0)
    use_mask = bool(np.any(attn_mask))

    weights = {
        "WQT": np.ascontiguousarray(W_Q.T.astype(bf)),
        "WKT": np.ascontiguousarray(W_K.T.astype(bf)),
        "WVT": np.ascontiguousarray(W_V.T.astype(bf)),
        "WFCT": np.ascontiguousarray(W_fc.T.astype(bf)),
    }
    tw_extra = {}
    if use_tw:
        # twT[kt, qt] for head h at rows 64*(h%2), cols 50*h
        twt = np.zeros((2 * SPAD, H * S), dtype=bf)
        twtt = time_weight.transpose(0, 2, 1)  # [h, kt, qt]
        for h in range(H):
            r0 = SPAD * (h % 2)
            twt[r0:r0 + S, h * S:(h + 1) * S] = twtt[h].astype(bf)
        tw_extra["TWT"] = twt

    nc = _get_nc(NCHUNK, use_tw, use_mask)

    in_maps = []
    for c in range(NCORES):
        sl = slice(c * BC, (c + 1) * BC)
        mask_extra = {}
        if use_mask:
            mneg = np.zeros((BC, 2 * SPAD, H * S), dtype=np.float32)
            mc = attn_mask[sl]  # [BC, qt, kt]
            mt = np.where(mc, np.float32(NEG * 8.0), np.float32(0.0))
            # scoresT layout: [kt, qt]; exp scale=1/8 applied after add
            mtt = mt.transpose(0, 2, 1)  # [b, kt, qt]
            for b in range(BC):
                for h in range(H):
                    r0 = SPAD * (h % 2)
                    mneg[b, r0:r0 + S, h * S:(h + 1) * S] = mtt[b]
            mask_extra["MASKNEG"] = mneg
        in_maps.append(
            _prep_core_inputs(Q[sl], K[sl], V[sl], NCHUNK, weights,
                              tw_extra, mask_extra))

    res = run_bass_kernel_spmd(nc, in_maps, list(range(NCORES)),
                               trace=PROFILE, **PROFILE_KW)
    global LAST_RESULT
    LAST_RESULT = res
    out = np.empty((B, S, D), dtype=np.float32)
    for c in range(NCORES):
        out[c * BC:(c + 1) * BC] = res.results[c]["OUT"].reshape(BC, S, D)
    return out


PROFILE = False
PROFILE_KW = {}
LAST_RESULT = None


# revision 22
# speedup vs baseline: 1.2911x; 1.2911x over previous
"""Trainium2 Bass kernel for batched MultiHeadAttention + LayerNorm.

Computes, per batch element b (B=1024, S=50, D=512, H=8 heads, Dk=64):
    q = Q @ W_Q.T ; k = K @ W_K.T ; v = V @ W_V.T          (per-head split)
    scores = q k^T / sqrt(Dk)  (+ mask)   attn = softmax(scores) * time_weight
    context = attn @ v ;  out = LayerNorm(context @ W_fc.T + Q)

Sharding: data-parallel over batch across 8 NeuronCores (128 batches/core).
Weights replicated. All matmuls in bf16 (PE cannot LDWEIGHTS fp32); fp32
accumulation in PSUM; LayerNorm in fp32.

Device layout strategy (per core):
  - Q,K shipped feature-major (QT/KT [512, 6400] bf16) so the d-contraction
    projections need no on-device transpose.
  - V and residual-Q shipped feature-major/token-major with each batch padded
    to 64 rows (t = b*64 + s) so per-batch rows are 64-aligned for PE
    partition slicing.
  - scores computed transposed (scoresT[kt, qt] = k q^T): softmax sum over kt
    is done on the PE (expT @ ones), and expT is directly the stationary
    operand of the context matmul -> no attention transposes.
  - context comes out token-major [qt, e]; normalization by 1/rowsum is fused
    into the PSUM->SBUF evacuation multiply; DMA-xbar transposes (bf16) give
    the feature-major tiles for the final projection.
  - residual add is a PE matmul with the identity (accumulates into the same
    PSUM bank as the final projection); LayerNorm via bn_stats/bn_aggr +
    fused (x - mu) * rstd tensor_scalar.
"""

import sys

sys.path.insert(0, "/opt/trn_rl_repo")

import numpy as np
import ml_dtypes

import concourse.bass as bass
import concourse.tile as tile
from concourse import bacc, mybir
from concourse.bass_utils import run_bass_kernel_spmd
from concourse.masks import make_identity

BF16 = mybir.dt.bfloat16
FP32 = mybir.dt.float32

B = 1024
S = 50
D = 512
H = 8
DK = 64
NCORES = 8
BC = B // NCORES          # batches per core = 128
SPAD = 64                 # padded seq rows per batch
NB = 16                   # batches per chunk
NCHUNK = BC // NB         # 8 chunks
CT = NB * S               # real tokens per chunk = 800
CTP = NB * SPAD           # padded tokens per chunk = 1024
T = BC * S                # real tokens per core = 6400
TP = BC * SPAD            # padded tokens per core = 8192
NEG = -1e9
LN_EPS = 1e-5
INV_SQRT_DK = 1.0 / 8.0


def _build_nc(n_chunk=NCHUNK, use_tw=False, use_mask=False, stage=9):
    nc = bacc.Bacc("TRN2", target_bir_lowering=False, debug=False)

    t_tot = n_chunk * CT
    tp_tot = n_chunk * CTP
    qt_d = nc.dram_tensor("QT", [D, t_tot], BF16, kind="ExternalInput")
    kt_d = nc.dram_tensor("KT", [D, t_tot], BF16, kind="ExternalInput")
    vt_d = nc.dram_tensor("VT", [D, tp_tot], BF16, kind="ExternalInput")
    qres_d = nc.dram_tensor("QRES", [tp_tot, D], BF16, kind="ExternalInput")
    wqt_d = nc.dram_tensor("WQT", [D, D], BF16, kind="ExternalInput")
    wkt_d = nc.dram_tensor("WKT", [D, D], BF16, kind="ExternalInput")
    wvt_d = nc.dram_tensor("WVT", [D, D], BF16, kind="ExternalInput")
    wfct_d = nc.dram_tensor("WFCT", [D, D], BF16, kind="ExternalInput")
    out_d = nc.dram_tensor("OUT", [t_tot, D], FP32, kind="ExternalOutput")
    if use_tw:
        # twT[h] at rows 64*(h%2), cols 50*h: [128, H*S]
        twt_d = nc.dram_tensor("TWT", [2 * SPAD, H * S], BF16, kind="ExternalInput")
    if use_mask:
        # additive mask (pre-scaled by sqrt(dk)), scoresT layout per batch,
        # head h block at rows 64*(h%2), cols 50*h
        mask_d = nc.dram_tensor(
            "MASKNEG", [n_chunk * NB, 2 * SPAD, H * S], FP32,
            kind="ExternalInput",
        )

    from contextlib import ExitStack
    with tile.TileContext(nc) as tc, ExitStack() as ctx:
        consts = ctx.enter_context(tc.tile_pool(name="consts", bufs=1))
        sb = ctx.enter_context(tc.tile_pool(name="sb", bufs=1))
        ps = ctx.enter_context(tc.tile_pool(name="ps", bufs=1, space="PSUM"))

        # ---- persistent constants ----
        wq_sb = []
        wk_sb = []
        wv_sb = []
        wfc_sb = []
        for dt in range(4):
            t_q = consts.tile([128, D], BF16, name=f"wq{dt}", tag=f"wq{dt}")
            nc.sync.dma_start(out=t_q[:, :], in_=wqt_d[128 * dt:128 * (dt + 1), :])
            wq_sb.append(t_q)
            t_k = consts.tile([128, D], BF16, name=f"wk{dt}", tag=f"wk{dt}")
            nc.sync.dma_start(out=t_k[:, :], in_=wkt_d[128 * dt:128 * (dt + 1), :])
            wk_sb.append(t_k)
            t_v = consts.tile([128, D], BF16, name=f"wv{dt}", tag=f"wv{dt}")
            nc.sync.dma_start(out=t_v[:, :], in_=wvt_d[128 * dt:128 * (dt + 1), :])
            wv_sb.append(t_v)
            t_f = consts.tile([128, D], BF16, name=f"wfc{dt}", tag=f"wfc{dt}")
            nc.sync.dma_start(out=t_f[:, :], in_=wfct_d[128 * dt:128 * (dt + 1), :])
            wfc_sb.append(t_f)
        ident = consts.tile([128, 128], BF16, name="ident", tag="ident")
        make_identity(nc, ident[:, :])
        ones_sb = consts.tile([128, 1], BF16, name="ones", tag="ones")
        nc.vector.memset(ones_sb[:, :], 1.0)
        eps_sb = consts.tile([128, 1], FP32, name="eps", tag="eps")
        nc.vector.memset(eps_sb[:, :], LN_EPS)
        if use_tw:
            tw_sb = consts.tile([2 * SPAD, H * S], BF16, name="twsb", tag="twsb")
            nc.sync.dma_start(out=tw_sb[:, :], in_=twt_d[:, :])

        for ch in range(n_chunk):
            t0 = ch * CT          # real-token base of chunk
            tp0 = ch * CTP        # padded-token base of chunk

            # ---- load chunk inputs ----
            qtin = []
            ktin = []
            vtin = []
            for dt in range(4):
                ti_q = sb.tile([128, CT], BF16, name=f"qtin{dt}", tag="qtin", bufs=8)
                nc.sync.dma_start(
                    out=ti_q[:, :], in_=qt_d[128 * dt:128 * (dt + 1), t0:t0 + CT])
                qtin.append(ti_q)
                ti_k = sb.tile([128, CT], BF16, name=f"ktin{dt}", tag="ktin", bufs=8)
                nc.sync.dma_start(
                    out=ti_k[:, :], in_=kt_d[128 * dt:128 * (dt + 1), t0:t0 + CT])
                ktin.append(ti_k)
                ti_v = sb.tile([128, CTP], BF16, name=f"vtin{dt}", tag="vtin", bufs=8)
                nc.sync.dma_start(
                    out=ti_v[:, :], in_=vt_d[128 * dt:128 * (dt + 1), tp0:tp0 + CTP])
                vtin.append(ti_v)

            # ---- q/k projections -> feature-major qT/kT [4][128, CT] ----
            qT = []
            kT = []
            for et in range(4):
                tq = sb.tile([128, CT], BF16, name=f"qT{et}", tag="qT", bufs=8)
                tk = sb.tile([128, CT], BF16, name=f"kT{et}", tag="kT", bufs=8)
                for half in range(2):
                    hs = CT // 2
                    pq = ps.tile([128, hs], FP32, name="psq", tag="psA", bufs=2)
                    for dt in range(4):
                        nc.tensor.matmul(
                            pq[:, :],
                            lhsT=wq_sb[dt][:, 128 * et:128 * (et + 1)],
                            rhs=qtin[dt][:, half * hs:(half + 1) * hs],
                            start=(dt == 0), stop=(dt == 3),
                        )
                    nc.any.tensor_copy(tq[:, half * hs:(half + 1) * hs], pq[:, :])
                    pk = ps.tile([128, hs], FP32, name="psk", tag="psA", bufs=2)
                    for dt in range(4):
                        nc.tensor.matmul(
                            pk[:, :],
                            lhsT=wk_sb[dt][:, 128 * et:128 * (et + 1)],
                            rhs=ktin[dt][:, half * hs:(half + 1) * hs],
                            start=(dt == 0), stop=(dt == 3),
                        )
                    nc.any.tensor_copy(tk[:, half * hs:(half + 1) * hs], pk[:, :])
                qT.append(tq)
                kT.append(tk)

            # ---- v projection -> token-major padded v [NB//2][128, 512] ----
            v_c = []
            for tt in range(NB // 2):
                pv = ps.tile([128, D], FP32, name="psv", tag="psA", bufs=2)
                for dt in range(4):
                    nc.tensor.matmul(
                        pv[:, :],
                        lhsT=vtin[dt][:, 128 * tt:128 * (tt + 1)],
                        rhs=wv_sb[dt][:, :],
                        start=(dt == 0), stop=(dt == 3),
                    )
                tv = sb.tile([128, D], BF16, name=f"v{tt}", tag="vc", bufs=16)
                nc.any.tensor_copy(tv[:, :], pv[:, :])
                v_c.append(tv)

            # parity-swapped copy of v (batch rows moved to the other 64-block)
            v2_c = []
            for tt in range(NB // 2):
                tv2 = sb.tile([128, D], BF16, name=f"v2{tt}", tag="v2c", bufs=16)
                nc.sync.dma_start(out=tv2[0:64, :], in_=v_c[tt][64:128, :])
                nc.sync.dma_start(out=tv2[64:128, :], in_=v_c[tt][0:64, :])
                v2_c.append(tv2)

            if stage < 2:
                continue
            # ---- attention + output ----
            # scoresT layout per batch: head h=2*hh+hp lives at partition rows
            # 64*hp, free cols 200*hp + 50*hh. All tile_positions diagonal or
            # (0, 64): (row>col) hangs the PE.
            mvall = sb.tile([128, NB], FP32, name="mvall", tag="mvall",
                            bufs=2)
            out1_list = []
            for bp in range(NB // 2):
                cT_pair = []
                for p in range(2):
                    b_loc = 2 * bp + p
                    tloc = S * b_loc
                    psc = ps.tile([128, 512], FP32, name="psc", tag="psB",
                                  bufs=3)
                    expT = sb.tile([128, 400], BF16, name="expT", tag="expT",
                                   bufs=4)
                    if use_mask:
                        bi = ch * NB + b_loc
                        mrow = sb.tile([128, 400], FP32, name="mrow",
                                       tag="mrow", bufs=4)
                        nc.sync.dma_start(out=mrow[:, :], in_=mask_d[bi, :, :])
                    for h in range(8):
                        hp, hh = h % 2, h // 2
                        col = 200 * hp + 50 * hh
                        nc.tensor.matmul(
                            psc[64 * hp:64 * hp + S, col:col + S],
                            lhsT=kT[h // 2][64 * hp:64 * hp + 64, tloc:tloc + S],
                            rhs=qT[h // 2][64 * hp:64 * hp + 64, tloc:tloc + S],
                            start=True, stop=True,
                            tile_position=(64 * hp, 64 * hp),
                        )
                    if stage < 3:
                        continue
                    for hp in range(2):
                        rows = slice(64 * hp, 64 * hp + S)
                        cols = slice(200 * hp, 200 * hp + 200)
                        if use_mask:
                            nc.vector.tensor_add(
                                psc[rows, cols], psc[rows, cols],
                                mrow[rows, cols])
                        nc.scalar.activation(
                            out=expT[rows, cols], in_=psc[rows, cols],
                            func=mybir.ActivationFunctionType.Exp,
                            scale=INV_SQRT_DK,
                        )
                        if use_tw:
                            nc.vector.tensor_mul(
                                expT[rows, cols], expT[rows, cols],
                                tw_sb[rows, cols])
                    if stage < 4:
                        continue
                    # row sums (transposed): srow[64*hp, 200*hp+50*hh+qt]
                    srow = ps.tile([128, 512], FP32, name="srow", tag="sr",
                                   bufs=2)
                    for h in range(8):
                        hp, hh = h % 2, h // 2
                        col = 200 * hp + 50 * hh
                        nc.tensor.matmul(
                            srow[64 * hp:64 * hp + 1, col:col + S],
                            lhsT=ones_sb[64 * hp:64 * hp + S, :],
                            rhs=expT[64 * hp:64 * hp + S, col:col + S],
                            start=True, stop=True,
                            tile_position=(64 * hp, 64 * hp),
                        )
                    # reciprocal row-sums assembled into one partition-0
                    # row [1, 400] (even | odd), then one full broadcast
                    # (partition_broadcast writes garbage for out offset 64)
                    rrow = sb.tile([128, 400], FP32, name="rrow", tag="rrow",
                                   bufs=4)
                    rcomb = sb.tile([1, 400], FP32, name="rcomb", tag="rcomb",
                                    bufs=4)
                    nc.vector.reciprocal(rcomb[0:1, 0:200], srow[0:1, 0:200])
                    nc.vector.reciprocal(rrow[64:65, 200:400],
                                         srow[64:65, 200:400])
                    nc.sync.dma_start(out=rcomb[0:1, 200:400],
                                      in_=rrow[64:65, 200:400])
                    rexp2 = sb.tile([128, 400], FP32, name="rexp2",
                                    tag="rexp2", bufs=4)
                    nc.gpsimd.partition_broadcast(rexp2[:, :], rcomb[0:1, :])
                    if stage < 5:
                        continue
                    # context, feature-major: ctxT[128=e of (h=2j,2j+1), 50*j+qt]
                    ctxT = ps.tile([128, 512], FP32, name="ctxT", tag="psB",
                                   bufs=3)
                    for h in range(8):
                        hp, hh = h % 2, h // 2
                        col = 200 * hp + 50 * hh
                        vsrc = v_c[bp] if hp == p else v2_c[bp]
                        nc.tensor.matmul(
                            ctxT[64 * hp:64 * hp + 64, 50 * hh:50 * hh + S],
                            lhsT=vsrc[64 * hp:64 * hp + S,
                                      DK * h:DK * (h + 1)],
                            rhs=expT[64 * hp:64 * hp + S, col:col + S],
                            start=True, stop=True,
                            tile_position=(64 * hp, 64 * hp),
                        )
                    if stage < 6:
                        continue
                    # normalize + evacuate (bf16): cT[e_row, 50*j+qt]
                    cT_sb = sb.tile([128, 200], BF16, name="ctsb", tag="ctsb",
                                    bufs=4)
                    nc.vector.tensor_mul(cT_sb[0:64, :], ctxT[0:64, 0:200],
                                         rexp2[0:64, 0:200])
                    nc.vector.tensor_mul(cT_sb[64:128, :],
                                         ctxT[64:128, 0:200],
                                         rexp2[64:128, 200:400])
                    cT_pair.append(cT_sb)

                if stage < 7:
                    continue
                # residual tile
                qres_sb = sb.tile([128, D], BF16, name="qres", tag="qres",
                                  bufs=3)
                nc.sync.dma_start(
                    out=qres_sb[:, :],
                    in_=qres_d[tp0 + 128 * bp:tp0 + 128 * (bp + 1), :])

                # residual first (opens the PSUM group over all rows),
                # then the final projection accumulates onto it
                po = ps.tile([128, D], FP32, name="po", tag="psD", bufs=1)
                nc.tensor.matmul(
                    po[:, :], lhsT=ident[:, :], rhs=qres_sb[:, :],
                    start=True, stop=False, skip_group_check=True,
                )
                for p in range(2):
                    for j in range(4):
                        nc.tensor.matmul(
                            po[64 * p:64 * p + S, :],
                            lhsT=cT_pair[p][:, 50 * j:50 * j + S],
                            rhs=wfc_sb[j][:, :],
                            start=False, stop=(p == 1 and j == 3),
                            tile_position=(0, 64 * p), skip_group_check=True,
                        )

                if stage < 9:
                    continue
                # LayerNorm stats; sqrt is batched once per chunk (ACT table)
                stats = sb.tile([128, 6], FP32, name="stats", tag="stats",
                                bufs=4)
                nc.vector.bn_stats(out=stats[:, :], in_=po[:, :])
                nc.vector.bn_aggr(out=mvall[:, 2 * bp:2 * bp + 2],
                                  in_=stats[:, :])
                out1 = sb.tile([128, D], FP32, name="out1", tag="out1",
                               bufs=NB // 2 + 1)
                nc.vector.tensor_scalar(
                    out=out1[:, :], in0=po[:, :],
                    scalar1=mvall[:, 2 * bp:2 * bp + 1], scalar2=None,
                    op0=mybir.AluOpType.subtract,
                )
                out1_list.append(out1)

            if stage < 9:
                continue
            # batched rstd + final scale + store
            stdall = sb.tile([128, NB // 2], FP32, name="stdall", tag="stdall",
                             bufs=2)
            rstdall = sb.tile([128, NB // 2], FP32, name="rstdall",
                              tag="rstdall", bufs=2)
            nc.scalar.activation(
                out=stdall[:, :],
                in_=mvall[:, :].rearrange("p (b two) -> p two b", two=2)[:, 1, :],
                func=mybir.ActivationFunctionType.Sqrt,
                bias=eps_sb[:, :])
            nc.vector.reciprocal(rstdall[:, :], stdall[:, :])
            for bp in range(NB // 2):
                out_sb = sb.tile([128, D], FP32, name="outsb", tag="outsb",
                                 bufs=3)
                nc.vector.tensor_scalar_mul(
                    out_sb[:, :], out1_list[bp][:, :],
                    rstdall[:, bp:bp + 1])
                b0 = ch * NB + 2 * bp
                nc.sync.dma_start(
                    out=out_d[S * b0:S * (b0 + 1), :], in_=out_sb[0:S, :])
                nc.sync.dma_start(
                    out=out_d[S * (b0 + 1):S * (b0 + 2), :],
                    in_=out_sb[64:64 + S, :])

    nc.compile()
    return nc


_NC_CACHE = {}


def _get_nc(n_chunk, use_tw, use_mask):
    key = (n_chunk, use_tw, use_mask)
    if key not in _NC_CACHE:
        _NC_CACHE[key] = _build_nc(n_chunk, use_tw, use_mask)
    return _NC_CACHE[key]


def _prep_core_inputs(Qc, Kc, Vc, n_chunk, weights, tw_extra, mask_extra):
    """Host-side layout prep for one core's shard (nb batches)."""
    nb = Qc.shape[0]
    t = nb * S
    tp = nb * SPAD
    bf = ml_dtypes.bfloat16
    qt = np.ascontiguousarray(Qc.reshape(t, D).T.astype(bf))
    kt = np.ascontiguousarray(Kc.reshape(t, D).T.astype(bf))
    vp = np.zeros((nb, SPAD, D), dtype=bf)
    vp[:, :S] = Vc
    vt = np.ascontiguousarray(vp.reshape(tp, D).T)
    qres = np.zeros((nb, SPAD, D), dtype=bf)
    qres[:, :S] = Qc
    qres = qres.reshape(tp, D)
    m = {"QT": qt, "KT": kt, "VT": vt, "QRES": qres}
    m.update(weights)
    m.update(tw_extra)
    m.update(mask_extra)
    return m


def kernel(Q, K, V, attn_mask, W_Q, W_K, W_V, W_fc, time_weight):
    Q = np.asarray(Q, dtype=np.float32)
    K = np.asarray(K, dtype=np.float32)
    V = np.asarray(V, dtype=np.float32)
    W_Q = np.asarray(W_Q, dtype=np.float32)
    W_K = np.asarray(W_K, dtype=np.float32)
    W_V = np.asarray(W_V, dtype=np.float32)
    W_fc = np.asarray(W_fc, dtype=np.float32)
    time_weight = np.asarray(time_weight, dtype=np.float32)
    attn_mask = np.asarray(attn_mask)

    bf = ml_dtypes.bfloat16
    use_tw = not np.all(time_weight == 1.0)
    use_mask = bool(np.any(attn_mask))

    weights = {
        "WQT": np.ascontiguousarray(W_Q.T.astype(bf)),
        "WKT": np.ascontiguousarray(W_K.T.astype(bf)),
        "WVT": np.ascontiguousarray(W_V.T.astype(bf)),
        "WFCT": np.ascontiguousarray(W_fc.T.astype(bf)),
    }
    tw_extra = {}
    if use_tw:
        # twT[kt, qt] for head h at rows 64*(h%2), cols 50*h
        twt = np.zeros((2 * SPAD, H * S), dtype=bf)
        twtt = time_weight.transpose(0, 2, 1)  # [h, kt, qt]
        for h in range(H):
            r0 = SPAD * (h % 2)
            c0 = 200 * (h % 2) + 50 * (h // 2)
            twt[r0:r0 + S, c0:c0 + S] = twtt[h].astype(bf)
        tw_extra["TWT"] = twt

    nc = _get_nc(NCHUNK, use_tw, use_mask)

    in_maps = []
    for c in range(NCORES):
        sl = slice(c * BC, (c + 1) * BC)
        mask_extra = {}
        if use_mask:
            mneg = np.zeros((BC, 2 * SPAD, H * S), dtype=np.float32)
            mc = attn_mask[sl]  # [BC, qt, kt]
            mt = np.where(mc, np.float32(NEG * 8.0), np.float32(0.0))
            # scoresT layout: [kt, qt]; exp scale=1/8 applied after add
            mtt = mt.transpose(0, 2, 1)  # [b, kt, qt]
            for b in range(BC):
                for h in range(H):
                    r0 = SPAD * (h % 2)
                    c0 = 200 * (h % 2) + 50 * (h // 2)
                    mneg[b, r0:r0 + S, c0:c0 + S] = mtt[b]
            mask_extra["MASKNEG"] = mneg
        in_maps.append(
            _prep_core_inputs(Q[sl], K[sl], V[sl], NCHUNK, weights,
                              tw_extra, mask_extra))

    res = run_bass_kernel_spmd(nc, in_maps, list(range(NCORES)),
                               trace=PROFILE, **PROFILE_KW)
    global LAST_RESULT
    LAST_RESULT = res
    out = np.empty((B, S, D), dtype=np.float32)
    for c in range(NCORES):
        out[c * BC:(c + 1) * BC] = res.results[c]["OUT"].reshape(BC, S, D)
    return out


PROFILE = False
PROFILE_KW = {}
LAST_RESULT = None


# revision 25
# speedup vs baseline: 1.4702x; 1.1387x over previous
"""Trainium2 Bass kernel for batched MultiHeadAttention + LayerNorm.

Computes, per batch element b (B=1024, S=50, D=512, H=8 heads, Dk=64):
    q = Q @ W_Q.T ; k = K @ W_K.T ; v = V @ W_V.T          (per-head split)
    scores = q k^T / sqrt(Dk)  (+ mask)   attn = softmax(scores) * time_weight
    context = attn @ v ;  out = LayerNorm(context @ W_fc.T + Q)

Sharding: data-parallel over batch across 8 NeuronCores (128 batches/core).
Weights replicated. All matmuls in bf16 (PE cannot LDWEIGHTS fp32); fp32
accumulation in PSUM; LayerNorm in fp32.

Device layout strategy (per core):
  - Q,K shipped feature-major (QT/KT [512, 6400] bf16) so the d-contraction
    projections need no on-device transpose.
  - V and residual-Q shipped feature-major/token-major with each batch padded
    to 64 rows (t = b*64 + s) so per-batch rows are 64-aligned for PE
    partition slicing.
  - scores computed transposed (scoresT[kt, qt] = k q^T): softmax sum over kt
    is done on the PE (expT @ ones), and expT is directly the stationary
    operand of the context matmul -> no attention transposes.
  - context comes out token-major [qt, e]; normalization by 1/rowsum is fused
    into the PSUM->SBUF evacuation multiply; DMA-xbar transposes (bf16) give
    the feature-major tiles for the final projection.
  - residual add is a PE matmul with the identity (accumulates into the same
    PSUM bank as the final projection); LayerNorm via bn_stats/bn_aggr +
    fused (x - mu) * rstd tensor_scalar.
"""

import sys

sys.path.insert(0, "/opt/trn_rl_repo")

import numpy as np
import ml_dtypes

import concourse.bass as bass
import concourse.tile as tile
from concourse import bacc, mybir
from concourse.bass_utils import run_bass_kernel_spmd
from concourse.masks import make_identity

BF16 = mybir.dt.bfloat16
FP32 = mybir.dt.float32

B = 1024
S = 50
D = 512
H = 8
DK = 64
NCORES = 8
BC = B // NCORES          # batches per core = 128
SPAD = 64                 # padded seq rows per batch
NB = 16                   # batches per chunk
NCHUNK = BC // NB         # 8 chunks
CT = NB * S               # real tokens per chunk = 800
CTP = NB * SPAD           # padded tokens per chunk = 1024
T = BC * S                # real tokens per core = 6400
TP = BC * SPAD            # padded tokens per core = 8192
NEG = -1e9
LN_EPS = 1e-5
INV_SQRT_DK = 1.0 / 8.0


def _build_nc(n_chunk=NCHUNK, use_tw=False, use_mask=False, stage=9):
    nc = bacc.Bacc("TRN2", target_bir_lowering=False, debug=False)

    t_tot = n_chunk * CT
    tp_tot = n_chunk * CTP
    qt_d = nc.dram_tensor("QT", [D, t_tot], BF16, kind="ExternalInput")
    kt_d = nc.dram_tensor("KT", [D, t_tot], BF16, kind="ExternalInput")
    vt_d = nc.dram_tensor("VT", [D, tp_tot], BF16, kind="ExternalInput")
    qres_d = nc.dram_tensor("QRES", [tp_tot, D], BF16, kind="ExternalInput")
    wqt_d = nc.dram_tensor("WQT", [D, D], BF16, kind="ExternalInput")
    wkt_d = nc.dram_tensor("WKT", [D, D], BF16, kind="ExternalInput")
    wvt_d = nc.dram_tensor("WVT", [D, D], BF16, kind="ExternalInput")
    wfct_d = nc.dram_tensor("WFCT", [D, D], BF16, kind="ExternalInput")
    out_d = nc.dram_tensor("OUT", [t_tot, D], FP32, kind="ExternalOutput")
    if use_tw:
        # twT[h] at rows 64*(h%2), cols 50*h: [128, H*S]
        twt_d = nc.dram_tensor("TWT", [2 * SPAD, H * S], BF16, kind="ExternalInput")
    if use_mask:
        # additive mask (pre-scaled by sqrt(dk)), scoresT layout per batch,
        # head h block at rows 64*(h%2), cols 50*h
        mask_d = nc.dram_tensor(
            "MASKNEG", [n_chunk * NB, 2 * SPAD, H * S], FP32,
            kind="ExternalInput",
        )

    from contextlib import ExitStack
    with tile.TileContext(nc) as tc, ExitStack() as ctx:
        consts = ctx.enter_context(tc.tile_pool(name="consts", bufs=1))
        sb = ctx.enter_context(tc.tile_pool(name="sb", bufs=1))
        ps = ctx.enter_context(tc.tile_pool(name="ps", bufs=1, space="PSUM"))

        # ---- persistent constants ----
        wq_sb = []
        wk_sb = []
        wv_sb = []
        wfc_sb = []
        for dt in range(4):
            t_q = consts.tile([128, D], BF16, name=f"wq{dt}", tag=f"wq{dt}")
            nc.sync.dma_start(out=t_q[:, :], in_=wqt_d[128 * dt:128 * (dt + 1), :])
            wq_sb.append(t_q)
            t_k = consts.tile([128, D], BF16, name=f"wk{dt}", tag=f"wk{dt}")
            nc.sync.dma_start(out=t_k[:, :], in_=wkt_d[128 * dt:128 * (dt + 1), :])
            wk_sb.append(t_k)
            t_v = consts.tile([128, D], BF16, name=f"wv{dt}", tag=f"wv{dt}")
            nc.sync.dma_start(out=t_v[:, :], in_=wvt_d[128 * dt:128 * (dt + 1), :])
            wv_sb.append(t_v)
            t_f = consts.tile([128, D], BF16, name=f"wfc{dt}", tag=f"wfc{dt}")
            nc.sync.dma_start(out=t_f[:, :], in_=wfct_d[128 * dt:128 * (dt + 1), :])
            wfc_sb.append(t_f)
        ident = consts.tile([128, 128], BF16, name="ident", tag="ident")
        make_identity(nc, ident[:, :])
        identf = consts.tile([128, 128], FP32, name="identf", tag="identf")
        make_identity(nc, identf[:, :])
        ones_sb = consts.tile([128, 1], BF16, name="ones", tag="ones")
        nc.vector.memset(ones_sb[:, :], 1.0)
        eps_sb = consts.tile([128, 1], FP32, name="eps", tag="eps")
        nc.vector.memset(eps_sb[:, :], LN_EPS)
        if use_tw:
            tw_sb = consts.tile([2 * SPAD, H * S], BF16, name="twsb", tag="twsb")
            nc.sync.dma_start(out=tw_sb[:, :], in_=twt_d[:, :])

        for ch in range(n_chunk):
            t0 = ch * CT          # real-token base of chunk
            tp0 = ch * CTP        # padded-token base of chunk

            # ---- load chunk inputs ----
            qtin = []
            ktin = []
            vtin = []
            for dt in range(4):
                ti_q = sb.tile([128, CT], BF16, name=f"qtin{dt}", tag="qtin", bufs=8)
                nc.sync.dma_start(
                    out=ti_q[:, :], in_=qt_d[128 * dt:128 * (dt + 1), t0:t0 + CT])
                qtin.append(ti_q)
                ti_k = sb.tile([128, CT], BF16, name=f"ktin{dt}", tag="ktin", bufs=8)
                nc.sync.dma_start(
                    out=ti_k[:, :], in_=kt_d[128 * dt:128 * (dt + 1), t0:t0 + CT])
                ktin.append(ti_k)
                ti_v = sb.tile([128, CTP], BF16, name=f"vtin{dt}", tag="vtin", bufs=8)
                nc.sync.dma_start(
                    out=ti_v[:, :], in_=vt_d[128 * dt:128 * (dt + 1), tp0:tp0 + CTP])
                vtin.append(ti_v)

            # ---- q/k projections -> feature-major qT/kT [4][128, CT] ----
            qT = []
            kT = []
            for et in range(4):
                tq = sb.tile([128, CT], BF16, name=f"qT{et}", tag="qT", bufs=8)
                tk = sb.tile([128, CT], BF16, name=f"kT{et}", tag="kT", bufs=8)
                for half in range(2):
                    hs = CT // 2
                    pq = ps.tile([128, hs], FP32, name="psq", tag="psA", bufs=2)
                    for dt in range(4):
                        nc.tensor.matmul(
                            pq[:, :],
                            lhsT=wq_sb[dt][:, 128 * et:128 * (et + 1)],
                            rhs=qtin[dt][:, half * hs:(half + 1) * hs],
                            start=(dt == 0), stop=(dt == 3),
                        )
                    nc.any.tensor_copy(tq[:, half * hs:(half + 1) * hs], pq[:, :])
                    pk = ps.tile([128, hs], FP32, name="psk", tag="psA", bufs=2)
                    for dt in range(4):
                        nc.tensor.matmul(
                            pk[:, :],
                            lhsT=wk_sb[dt][:, 128 * et:128 * (et + 1)],
                            rhs=ktin[dt][:, half * hs:(half + 1) * hs],
                            start=(dt == 0), stop=(dt == 3),
                        )
                    nc.any.tensor_copy(tk[:, half * hs:(half + 1) * hs], pk[:, :])
                qT.append(tq)
                kT.append(tk)

            # ---- v projection -> token-major padded v [NB//2][128, 512] ----
            v_c = []
            for tt in range(NB // 2):
                pv = ps.tile([128, D], FP32, name="psv", tag="psA", bufs=2)
                for dt in range(4):
                    nc.tensor.matmul(
                        pv[:, :],
                        lhsT=vtin[dt][:, 128 * tt:128 * (tt + 1)],
                        rhs=wv_sb[dt][:, :],
                        start=(dt == 0), stop=(dt == 3),
                    )
                tv = sb.tile([128, D], BF16, name=f"v{tt}", tag="vc", bufs=16)
                nc.any.tensor_copy(tv[:, :], pv[:, :])
                v_c.append(tv)

            # parity-swapped copy of v (batch rows moved to the other 64-block)
            v2_c = []
            for tt in range(NB // 2):
                tv2 = sb.tile([128, D], BF16, name=f"v2{tt}", tag="v2c", bufs=16)
                nc.sync.dma_start(out=tv2[0:64, :], in_=v_c[tt][64:128, :])
                nc.sync.dma_start(out=tv2[64:128, :], in_=v_c[tt][0:64, :])
                v2_c.append(tv2)

            if stage < 2:
                continue
            # ---- attention + output ----
            # scoresT layout per batch: head h=2*hh+hp lives at partition rows
            # 64*hp, free cols 200*hp + 50*hh. All tile_positions diagonal or
            # (0, 64): (row>col) hangs the PE.
            mvall = sb.tile([128, NB], FP32, name="mvall", tag="mvall",
                            bufs=2)
            out1_list = []
            for bp in range(NB // 2):
                cT_pair = []
                for p in range(2):
                    b_loc = 2 * bp + p
                    tloc = S * b_loc
                    psc = ps.tile([128, 512], FP32, name="psc", tag="psB",
                                  bufs=3)
                    expT = sb.tile([128, 400], BF16, name="expT", tag="expT",
                                   bufs=4)
                    if use_mask:
                        bi = ch * NB + b_loc
                        mrow = sb.tile([128, 400], FP32, name="mrow",
                                       tag="mrow", bufs=4)
                        nc.sync.dma_start(out=mrow[:, :], in_=mask_d[bi, :, :])
                    for h in range(8):
                        hp, hh = h % 2, h // 2
                        col = 200 * hp + 50 * hh
                        nc.tensor.matmul(
                            psc[64 * hp:64 * hp + S, col:col + S],
                            lhsT=kT[h // 2][64 * hp:64 * hp + 64, tloc:tloc + S],
                            rhs=qT[h // 2][64 * hp:64 * hp + 64, tloc:tloc + S],
                            start=True, stop=True,
                            tile_position=(64 * hp, 64 * hp),
                        )
                    if stage < 3:
                        continue
                    for hp in range(2):
                        rows = slice(64 * hp, 64 * hp + S)
                        cols = slice(200 * hp, 200 * hp + 200)
                        if use_mask:
                            nc.vector.tensor_add(
                                psc[rows, cols], psc[rows, cols],
                                mrow[rows, cols])
                        nc.scalar.activation(
                            out=expT[rows, cols], in_=psc[rows, cols],
                            func=mybir.ActivationFunctionType.Exp,
                            scale=INV_SQRT_DK,
                        )
                        if use_tw:
                            nc.vector.tensor_mul(
                                expT[rows, cols], expT[rows, cols],
                                tw_sb[rows, cols])
                    if stage < 4:
                        continue
                    # context, feature-major: ctxT[128=e of (h=2j,2j+1), 50*j+qt]
                    # plus per-(qt,h) row sums in cols 200..208 of the same tile
                    ctxT = ps.tile([128, 512], FP32, name="ctxT", tag="psB",
                                   bufs=3)
                    for h in range(8):
                        hp, hh = h % 2, h // 2
                        col = 200 * hp + 50 * hh
                        vsrc = v_c[bp] if hp == p else v2_c[bp]
                        nc.tensor.matmul(
                            ctxT[64 * hp:64 * hp + 64, 50 * hh:50 * hh + S],
                            lhsT=vsrc[64 * hp:64 * hp + S,
                                      DK * h:DK * (h + 1)],
                            rhs=expT[64 * hp:64 * hp + S, col:col + S],
                            start=True, stop=True,
                            tile_position=(64 * hp, 64 * hp),
                        )
                        nc.tensor.matmul(
                            ctxT[64 * hp:64 * hp + S, 200 + h:201 + h],
                            lhsT=expT[64 * hp:64 * hp + S, col:col + S],
                            rhs=ones_sb[64 * hp:64 * hp + S, :],
                            start=True, stop=True,
                            tile_position=(64 * hp, 64 * hp),
                        )
                    if stage < 5:
                        continue
                    # reciprocal on qt-partitions (cheap: 4 elems/lane), then
                    # PE-transpose the [128, 8] recip matrix to free-major,
                    # gather to a partition-0 row, broadcast to 128 rows
                    r_sb = sb.tile([128, 8], FP32, name="rsb", tag="rsb",
                                   bufs=4)
                    nc.vector.memset(r_sb[:, :], 0.0)
                    sview = ctxT[:, 200:208].rearrange(
                        "p (c two) -> p two c", two=2)
                    nc.vector.reciprocal(r_sb[0:S, 0:4], sview[0:S, 0, :])
                    nc.vector.reciprocal(r_sb[64:64 + S, 4:8],
                                         sview[64:64 + S, 1, :])
                    rT_ps = ps.tile([8, 128], FP32, name="rtps", tag="sr",
                                    bufs=2)
                    nc.tensor.transpose(rT_ps[:, :], r_sb[:, :], identf[:, :])
                    rT_sb = sb.tile([8, 128], FP32, name="rtsb", tag="rtsb",
                                    bufs=4)
                    nc.vector.tensor_copy(rT_sb[:, :], rT_ps[:, :])
                    # rT_sb[c, 64*hp + qt] = 1/S[h=...]: even heads 2*hh at
                    # cols 0:4 rows 0:50 -> rT rows 0..3 cols 0:50; odd heads
                    # at cols 4:8 rows 64:114 -> rT rows 4..7 cols 64:114
                    rcomb = sb.tile([1, 400], FP32, name="rcomb", tag="rcomb",
                                    bufs=4)
                    nc.sync.dma_start(
                        out=rcomb[0:1, 0:200].rearrange(
                            "p (hh s) -> p hh s", s=S),
                        in_=rT_sb[0:4, 0:S])
                    nc.sync.dma_start(
                        out=rcomb[0:1, 200:400].rearrange(
                            "p (hh s) -> p hh s", s=S),
                        in_=rT_sb[4:8, 64:64 + S])
                    rexp2 = sb.tile([128, 400], FP32, name="rexp2",
                                    tag="rexp2", bufs=4)
                    nc.gpsimd.partition_broadcast(rexp2[:, :], rcomb[0:1, :])
                    if stage < 6:
                        continue
                    # normalize + evacuate (bf16): cT[e_row, 50*j+qt]
                    cT_sb = sb.tile([128, 200], BF16, name="ctsb", tag="ctsb",
                                    bufs=4)
                    nc.vector.tensor_mul(cT_sb[0:64, :], ctxT[0:64, 0:200],
                                         rexp2[0:64, 0:200])
                    nc.vector.tensor_mul(cT_sb[64:128, :],
                                         ctxT[64:128, 0:200],
                                         rexp2[64:128, 200:400])
                    cT_pair.append(cT_sb)

                if stage < 7:
                    continue
                # residual tile
                qres_sb = sb.tile([128, D], BF16, name="qres", tag="qres",
                                  bufs=3)
                nc.sync.dma_start(
                    out=qres_sb[:, :],
                    in_=qres_d[tp0 + 128 * bp:tp0 + 128 * (bp + 1), :])

                # residual first (opens the PSUM group over all rows),
                # then the final projection accumulates onto it
                po = ps.tile([128, D], FP32, name="po", tag="psD", bufs=1)
                nc.tensor.matmul(
                    po[:, :], lhsT=ident[:, :], rhs=qres_sb[:, :],
                    start=True, stop=False, skip_group_check=True,
                )
                for p in range(2):
                    for j in range(4):
                        nc.tensor.matmul(
                            po[64 * p:64 * p + S, :],
                            lhsT=cT_pair[p][:, 50 * j:50 * j + S],
                            rhs=wfc_sb[j][:, :],
                            start=False, stop=(p == 1 and j == 3),
                            tile_position=(0, 64 * p), skip_group_check=True,
                        )

                if stage < 9:
                    continue
                # LayerNorm stats; sqrt is batched once per chunk (ACT table)
                stats = sb.tile([128, 6], FP32, name="stats", tag="stats",
                                bufs=4)
                nc.vector.bn_stats(out=stats[:, :], in_=po[:, :])
                nc.vector.bn_aggr(out=mvall[:, 2 * bp:2 * bp + 2],
                                  in_=stats[:, :])
                out1 = sb.tile([128, D], FP32, name="out1", tag="out1",
                               bufs=NB // 2 + 1)
                nc.vector.tensor_scalar(
                    out=out1[:, :], in0=po[:, :],
                    scalar1=mvall[:, 2 * bp:2 * bp + 1], scalar2=None,
                    op0=mybir.AluOpType.subtract,
                )
                out1_list.append(out1)

            if stage < 9:
                continue
            # batched rstd + final scale + store
            stdall = sb.tile([128, NB // 2], FP32, name="stdall", tag="stdall",
                             bufs=2)
            rstdall = sb.tile([128, NB // 2], FP32, name="rstdall",
                              tag="rstdall", bufs=2)
            nc.scalar.activation(
                out=stdall[:, :],
                in_=mvall[:, :].rearrange("p (b two) -> p two b", two=2)[:, 1, :],
                func=mybir.ActivationFunctionType.Sqrt,
                bias=eps_sb[:, :])
            nc.vector.reciprocal(rstdall[:, :], stdall[:, :])
            for bp in range(NB // 2):
                out_sb = sb.tile([128, D], FP32, name="outsb", tag="outsb",
                                 bufs=3)
                nc.vector.tensor_scalar_mul(
                    out_sb[:, :], out1_list[bp][:, :],
                    rstdall[:, bp:bp + 1])
                b0 = ch * NB + 2 * bp
                nc.sync.dma_start(
                    out=out_d[S * b0:S * (b0 + 1), :], in_=out_sb[0:S, :])
                nc.sync.dma_start(
                    out=out_d[S * (b0 + 1):S * (b0 + 2), :],
                    in_=out_sb[64:64 + S, :])

    nc.compile()
    return nc


_NC_CACHE = {}


def _get_nc(n_chunk, use_tw, use_mask):
    key = (n_chunk, use_tw, use_mask)
    if key not in _NC_CACHE:
        _NC_CACHE[key] = _build_nc(n_chunk, use_tw, use_mask)
    return _NC_CACHE[key]


def _prep_core_inputs(Qc, Kc, Vc, n_chunk, weights, tw_extra, mask_extra):
    """Host-side layout prep for one core's shard (nb batches)."""
    nb = Qc.shape[0]
    t = nb * S
    tp = nb * SPAD
    bf = ml_dtypes.bfloat16
    qt = np.ascontiguousarray(Qc.reshape(t, D).T.astype(bf))
    kt = np.ascontiguousarray(Kc.reshape(t, D).T.astype(bf))
    vp = np.zeros((nb, SPAD, D), dtype=bf)
    vp[:, :S] = Vc
    vt = np.ascontiguousarray(vp.reshape(tp, D).T)
    qres = np.zeros((nb, SPAD, D), dtype=bf)
    qres[:, :S] = Qc
    qres = qres.reshape(tp, D)
    m = {"QT": qt, "KT": kt, "VT": vt, "QRES": qres}
    m.update(weights)
    m.update(tw_extra)
    m.update(mask_extra)
    return m


def kernel(Q, K, V, attn_mask, W_Q, W_K, W_V, W_fc, time_weight):
    Q = np.asarray(Q, dtype=np.float32)
    K = np.asarray(K, dtype=np.float32)
    V = np.asarray(V, dtype=np.float32)
    W_Q = np.asarray(W_Q, dtype=np.float32)
    W_K = np.asarray(W_K, dtype=np.float32)
    W_V = np.asarray(W_V, dtype=np.float32)
    W_fc = np.asarray(W_fc, dtype=np.float32)
    time_weight = np.asarray(time_weight, dtype=np.float32)
    attn_mask = np.asarray(attn_mask)

    bf = ml_dtypes.bfloat16
    use_tw = not np.all(time_weight == 1.0)
    use_mask = bool(np.any(attn_mask))

    weights = {
        "WQT": np.ascontiguousarray(W_Q.T.astype(bf)),
        "WKT": np.ascontiguousarray(W_K.T.astype(bf)),
        "WVT": np.ascontiguousarray(W_V.T.astype(bf)),
        "WFCT": np.ascontiguousarray(W_fc.T.astype(bf)),
    }
    tw_extra = {}
    if use_tw:
        # twT[kt, qt] for head h at rows 64*(h%2), cols 50*h
        twt = np.zeros((2 * SPAD, H * S), dtype=bf)
        twtt = time_weight.transpose(0, 2, 1)  # [h, kt, qt]
        for h in range(H):
            r0 = SPAD * (h % 2)
            c0 = 200 * (h % 2) + 50 * (h // 2)
            twt[r0:r0 + S, c0:c0 + S] = twtt[h].astype(bf)
        tw_extra["TWT"] = twt

    nc = _get_nc(NCHUNK, use_tw, use_mask)

    in_maps = []
    for c in range(NCORES):
        sl = slice(c * BC, (c + 1) * BC)
        mask_extra = {}
        if use_mask:
            mneg = np.zeros((BC, 2 * SPAD, H * S), dtype=np.float32)
            mc = attn_mask[sl]  # [BC, qt, kt]
            mt = np.where(mc, np.float32(NEG * 8.0), np.float32(0.0))
            # scoresT layout: [kt, qt]; exp scale=1/8 applied after add
            mtt = mt.transpose(0, 2, 1)  # [b, kt, qt]
            for b in range(BC):
                for h in range(H):
                    r0 = SPAD * (h % 2)
                    c0 = 200 * (h % 2) + 50 * (h // 2)
                    mneg[b, r0:r0 + S, c0:c0 + S] = mtt[b]
            mask_extra["MASKNEG"] = mneg
        in_maps.append(
            _prep_core_inputs(Q[sl], K[sl], V[sl], NCHUNK, weights,
                              tw_extra, mask_extra))

    res = run_bass_kernel_spmd(nc, in_maps, list(range(NCORES)),
                               trace=PROFILE, **PROFILE_KW)
    global LAST_RESULT
    LAST_RESULT = res
    out = np.empty((B, S, D), dtype=np.float32)
    for c in range(NCORES):
        out[c * BC:(c + 1) * BC] = res.results[c]["OUT"].reshape(BC, S, D)
    return out


PROFILE = False
PROFILE_KW = {}
LAST_RESULT = None


# revision 28
# speedup vs baseline: 1.6404x; 1.1158x over previous
"""Trainium2 Bass kernel for batched MultiHeadAttention + LayerNorm.

Computes, per batch element b (B=1024, S=50, D=512, H=8 heads, Dk=64):
    q = Q @ W_Q.T ; k = K @ W_K.T ; v = V @ W_V.T          (per-head split)
    scores = q k^T / sqrt(Dk)  (+ mask)   attn = softmax(scores) * time_weight
    context = attn @ v ;  out = LayerNorm(context @ W_fc.T + Q)

Sharding: data-parallel over batch across 8 NeuronCores (128 batches/core).
Weights replicated. All matmuls in bf16 (PE cannot LDWEIGHTS fp32); fp32
accumulation in PSUM; LayerNorm in fp32.

Device layout strategy (per core):
  - Q,K shipped feature-major (QT/KT [512, 6400] bf16) so the d-contraction
    projections need no on-device transpose.
  - V and residual-Q shipped feature-major/token-major with each batch padded
    to 64 rows (t = b*64 + s) so per-batch rows are 64-aligned for PE
    partition slicing.
  - scores computed transposed (scoresT[kt, qt] = k q^T): softmax sum over kt
    is done on the PE (expT @ ones), and expT is directly the stationary
    operand of the context matmul -> no attention transposes.
  - context comes out token-major [qt, e]; normalization by 1/rowsum is fused
    into the PSUM->SBUF evacuation multiply; DMA-xbar transposes (bf16) give
    the feature-major tiles for the final projection.
  - residual add is a PE matmul with the identity (accumulates into the same
    PSUM bank as the final projection); LayerNorm via bn_stats/bn_aggr +
    fused (x - mu) * rstd tensor_scalar.
"""

import sys

sys.path.insert(0, "/opt/trn_rl_repo")

import numpy as np
import ml_dtypes

import concourse.bass as bass
import concourse.tile as tile
from concourse import bacc, mybir
from concourse.bass_utils import run_bass_kernel_spmd
from concourse.masks import make_identity

BF16 = mybir.dt.bfloat16
FP32 = mybir.dt.float32

B = 1024
S = 50
D = 512
H = 8
DK = 64
NCORES = 8
BC = B // NCORES          # batches per core = 128
SPAD = 64                 # padded seq rows per batch
NB = 16                   # batches per chunk
NCHUNK = BC // NB         # 8 chunks
CT = NB * S               # real tokens per chunk = 800
CTP = NB * SPAD           # padded tokens per chunk = 1024
T = BC * S                # real tokens per core = 6400
TP = BC * SPAD            # padded tokens per core = 8192
NEG = -1e9
LN_EPS = 1e-5
INV_SQRT_DK = 1.0 / 8.0


def _build_nc(n_chunk=NCHUNK, use_tw=False, use_mask=False, stage=9):
    nc = bacc.Bacc("TRN2", target_bir_lowering=False, debug=False)

    t_tot = n_chunk * CT
    tp_tot = n_chunk * CTP
    qt_d = nc.dram_tensor("QT", [D, t_tot], BF16, kind="ExternalInput")
    kt_d = nc.dram_tensor("KT", [D, t_tot], BF16, kind="ExternalInput")
    vt_d = nc.dram_tensor("VT", [D, tp_tot], BF16, kind="ExternalInput")
    qres_d = nc.dram_tensor("QRES", [t_tot, D], BF16, kind="ExternalInput")
    wqt_d = nc.dram_tensor("WQT", [D, D], BF16, kind="ExternalInput")
    wkt_d = nc.dram_tensor("WKT", [D, D], BF16, kind="ExternalInput")
    wvt_d = nc.dram_tensor("WVT", [D, D], BF16, kind="ExternalInput")
    wfct_d = nc.dram_tensor("WFCT", [D, D], BF16, kind="ExternalInput")
    out_d = nc.dram_tensor("OUT", [t_tot, D], FP32, kind="ExternalOutput")
    if use_tw:
        # twT[h] at rows 64*(h%2), cols 50*h: [128, H*S]
        twt_d = nc.dram_tensor("TWT", [2 * SPAD, H * S], BF16, kind="ExternalInput")
    if use_mask:
        # additive mask (pre-scaled by sqrt(dk)), scoresT layout per batch,
        # head h block at rows 64*(h%2), cols 50*h
        mask_d = nc.dram_tensor(
            "MASKNEG", [n_chunk * NB, 2 * SPAD, H * S], FP32,
            kind="ExternalInput",
        )

    from contextlib import ExitStack
    with tile.TileContext(nc) as tc, ExitStack() as ctx:
        consts = ctx.enter_context(tc.tile_pool(name="consts", bufs=1))
        sb = ctx.enter_context(tc.tile_pool(name="sb", bufs=1))
        ps = ctx.enter_context(tc.tile_pool(name="ps", bufs=1, space="PSUM"))

        # ---- persistent constants ----
        wq_sb = []
        wk_sb = []
        wv_sb = []
        wfc_sb = []
        for dt in range(4):
            t_q = consts.tile([128, D], BF16, name=f"wq{dt}", tag=f"wq{dt}")
            nc.sync.dma_start(out=t_q[:, :], in_=wqt_d[128 * dt:128 * (dt + 1), :])
            wq_sb.append(t_q)
            t_k = consts.tile([128, D], BF16, name=f"wk{dt}", tag=f"wk{dt}")
            nc.sync.dma_start(out=t_k[:, :], in_=wkt_d[128 * dt:128 * (dt + 1), :])
            wk_sb.append(t_k)
            t_v = consts.tile([128, D], BF16, name=f"wv{dt}", tag=f"wv{dt}")
            nc.sync.dma_start(out=t_v[:, :], in_=wvt_d[128 * dt:128 * (dt + 1), :])
            wv_sb.append(t_v)
            t_f = consts.tile([128, D], BF16, name=f"wfc{dt}", tag=f"wfc{dt}")
            nc.sync.dma_start(out=t_f[:, :], in_=wfct_d[128 * dt:128 * (dt + 1), :])
            wfc_sb.append(t_f)
        ident = consts.tile([128, 128], BF16, name="ident", tag="ident")
        make_identity(nc, ident[:, :])
        identf = consts.tile([128, 128], FP32, name="identf", tag="identf")
        make_identity(nc, identf[:, :])
        ones_sb = consts.tile([128, 1], BF16, name="ones", tag="ones")
        nc.vector.memset(ones_sb[:, :], 1.0)
        eps_sb = consts.tile([128, 1], FP32, name="eps", tag="eps")
        nc.vector.memset(eps_sb[:, :], LN_EPS)
        if use_tw:
            tw_sb = consts.tile([2 * SPAD, H * S], BF16, name="twsb", tag="twsb")
            nc.sync.dma_start(out=tw_sb[:, :], in_=twt_d[:, :])

        for ch in range(n_chunk):
            t0 = ch * CT          # real-token base of chunk
            tp0 = ch * CTP        # padded-token base of chunk

            # ---- load chunk inputs ----
            qtin = []
            ktin = []
            vtin = []
            for dt in range(4):
                ti_q = sb.tile([128, CT], BF16, name=f"qtin{dt}", tag="qtin", bufs=8)
                nc.sync.dma_start(
                    out=ti_q[:, :], in_=qt_d[128 * dt:128 * (dt + 1), t0:t0 + CT])
                qtin.append(ti_q)
                ti_k = sb.tile([128, CT], BF16, name=f"ktin{dt}", tag="ktin", bufs=8)
                nc.sync.dma_start(
                    out=ti_k[:, :], in_=kt_d[128 * dt:128 * (dt + 1), t0:t0 + CT])
                ktin.append(ti_k)
                ti_v = sb.tile([128, CTP], BF16, name=f"vtin{dt}", tag="vtin", bufs=8)
                nc.sync.dma_start(
                    out=ti_v[:, :], in_=vt_d[128 * dt:128 * (dt + 1), tp0:tp0 + CTP])
                vtin.append(ti_v)

            # ---- q/k projections -> feature-major qT/kT [4][128, CT] ----
            qT = []
            kT = []
            for et in range(4):
                tq = sb.tile([128, CT], BF16, name=f"qT{et}", tag="qT", bufs=8)
                tk = sb.tile([128, CT], BF16, name=f"kT{et}", tag="kT", bufs=8)
                for half in range(2):
                    hs = CT // 2
                    pq = ps.tile([128, hs], FP32, name="psq", tag="psA", bufs=3)
                    for dt in range(4):
                        nc.tensor.matmul(
                            pq[:, :],
                            lhsT=wq_sb[dt][:, 128 * et:128 * (et + 1)],
                            rhs=qtin[dt][:, half * hs:(half + 1) * hs],
                            start=(dt == 0), stop=(dt == 3),
                        )
                    nc.any.tensor_copy(tq[:, half * hs:(half + 1) * hs], pq[:, :])
                    pk = ps.tile([128, hs], FP32, name="psk", tag="psA", bufs=3)
                    for dt in range(4):
                        nc.tensor.matmul(
                            pk[:, :],
                            lhsT=wk_sb[dt][:, 128 * et:128 * (et + 1)],
                            rhs=ktin[dt][:, half * hs:(half + 1) * hs],
                            start=(dt == 0), stop=(dt == 3),
                        )
                    nc.any.tensor_copy(tk[:, half * hs:(half + 1) * hs], pk[:, :])
                qT.append(tq)
                kT.append(tk)

            # ---- v projection -> token-major padded v [NB//2][128, 512] ----
            v_c = []
            for tt in range(NB // 2):
                pv = ps.tile([128, D], FP32, name="psv", tag="psA", bufs=3)
                for dt in range(4):
                    nc.tensor.matmul(
                        pv[:, :],
                        lhsT=vtin[dt][:, 128 * tt:128 * (tt + 1)],
                        rhs=wv_sb[dt][:, :],
                        start=(dt == 0), stop=(dt == 3),
                    )
                tv = sb.tile([128, D], BF16, name=f"v{tt}", tag="vc", bufs=16)
                nc.any.tensor_copy(tv[:, :], pv[:, :])
                v_c.append(tv)

            # parity-swapped copy of v (batch rows moved to the other 64-block)
            v2_c = []
            for tt in range(NB // 2):
                tv2 = sb.tile([128, D], BF16, name=f"v2{tt}", tag="v2c", bufs=16)
                nc.sync.dma_start(out=tv2[0:64, :], in_=v_c[tt][64:128, :])
                nc.sync.dma_start(out=tv2[64:128, :], in_=v_c[tt][0:64, :])
                v2_c.append(tv2)

            if stage < 2:
                continue
            # ---- attention + output ----
            # scoresT layout per batch: head h=2*hh+hp lives at partition rows
            # 64*hp, free cols 200*hp + 50*hh. All tile_positions diagonal or
            # (0, 64): (row>col) hangs the PE.
            mvall = sb.tile([128, NB], FP32, name="mvall", tag="mvall",
                            bufs=2)
            out1_list = []
            for bp in range(NB // 2):
                cT_both = None
                for p in range(2):
                    b_loc = 2 * bp + p
                    tloc = S * b_loc
                    psc = ps.tile([128, 512], FP32, name="psc", tag="psB",
                                  bufs=3)
                    expT = sb.tile([128, 400], BF16, name="expT", tag="expT",
                                   bufs=4)
                    if use_mask:
                        bi = ch * NB + b_loc
                        mrow = sb.tile([128, 400], FP32, name="mrow",
                                       tag="mrow", bufs=4)
                        nc.sync.dma_start(out=mrow[:, :], in_=mask_d[bi, :, :])
                    for h in range(8):
                        hp, hh = h % 2, h // 2
                        col = 200 * hp + 50 * hh
                        nc.tensor.matmul(
                            psc[64 * hp:64 * hp + S, col:col + S],
                            lhsT=kT[h // 2][64 * hp:64 * hp + 64, tloc:tloc + S],
                            rhs=qT[h // 2][64 * hp:64 * hp + 64, tloc:tloc + S],
                            start=True, stop=True,
                            tile_position=(64 * hp, 64 * hp),
                        )
                    if stage < 3:
                        continue
                    for hp in range(2):
                        rows = slice(64 * hp, 64 * hp + S)
                        cols = slice(200 * hp, 200 * hp + 200)
                        if use_mask:
                            nc.vector.tensor_add(
                                psc[rows, cols], psc[rows, cols],
                                mrow[rows, cols])
                        nc.scalar.activation(
                            out=expT[rows, cols], in_=psc[rows, cols],
                            func=mybir.ActivationFunctionType.Exp,
                            scale=INV_SQRT_DK,
                        )
                        if use_tw:
                            nc.vector.tensor_mul(
                                expT[rows, cols], expT[rows, cols],
                                tw_sb[rows, cols])
                    if stage < 4:
                        continue
                    # context, feature-major: ctxT[128=e of (h=2j,2j+1), 50*j+qt]
                    # plus per-(qt,h) row sums in cols 200..208 of the same tile
                    ctxT = ps.tile([128, 512], FP32, name="ctxT", tag="psB",
                                   bufs=3)
                    for h in range(8):
                        hp, hh = h % 2, h // 2
                        col = 200 * hp + 50 * hh
                        vsrc = v_c[bp] if hp == p else v2_c[bp]
                        nc.tensor.matmul(
                            ctxT[64 * hp:64 * hp + 64, 50 * hh:50 * hh + S],
                            lhsT=vsrc[64 * hp:64 * hp + S,
                                      DK * h:DK * (h + 1)],
                            rhs=expT[64 * hp:64 * hp + S, col:col + S],
                            start=True, stop=True,
                            tile_position=(64 * hp, 64 * hp),
                        )
                        nc.tensor.matmul(
                            ctxT[64 * hp:64 * hp + S, 200 + h:201 + h],
                            lhsT=expT[64 * hp:64 * hp + S, col:col + S],
                            rhs=ones_sb[64 * hp:64 * hp + S, :],
                            start=True, stop=True,
                            tile_position=(64 * hp, 64 * hp),
                        )
                    if stage < 5:
                        continue
                    # reciprocal on qt-partitions (cheap: 4 elems/lane), then
                    # PE-transpose the [128, 8] recip matrix to free-major,
                    # gather to a partition-0 row, broadcast to 128 rows
                    r_sb = sb.tile([128, 8], FP32, name="rsb", tag="rsb",
                                   bufs=4)
                    nc.vector.memset(r_sb[:, :], 0.0)
                    sview = ctxT[:, 200:208].rearrange(
                        "p (c two) -> p two c", two=2)
                    nc.vector.reciprocal(r_sb[0:S, 0:4], sview[0:S, 0, :])
                    nc.vector.reciprocal(r_sb[64:64 + S, 4:8],
                                         sview[64:64 + S, 1, :])
                    rT_ps = ctxT[0:8, 208:336]
                    nc.tensor.transpose(rT_ps, r_sb[:, :], identf[:, :])
                    rT_sb = sb.tile([8, 128], FP32, name="rtsb", tag="rtsb",
                                    bufs=4)
                    nc.vector.tensor_copy(rT_sb[:, :], rT_ps)
                    # rT_sb[c, 64*hp + qt] = 1/S[h=...]: even heads 2*hh at
                    # cols 0:4 rows 0:50 -> rT rows 0..3 cols 0:50; odd heads
                    # at cols 4:8 rows 64:114 -> rT rows 4..7 cols 64:114
                    rcomb = sb.tile([1, 400], FP32, name="rcomb", tag="rcomb",
                                    bufs=4)
                    nc.sync.dma_start(
                        out=rcomb[0:1, 0:200].rearrange(
                            "p (hh s) -> p hh s", s=S),
                        in_=rT_sb[0:4, 0:S])
                    nc.sync.dma_start(
                        out=rcomb[0:1, 200:400].rearrange(
                            "p (hh s) -> p hh s", s=S),
                        in_=rT_sb[4:8, 64:64 + S])
                    rexp2 = sb.tile([128, 400], FP32, name="rexp2",
                                    tag="rexp2", bufs=4)
                    nc.gpsimd.partition_broadcast(rexp2[:, :], rcomb[0:1, :])
                    if stage < 6:
                        continue
                    # normalize + evacuate (bf16): cT[e_row, 100*j+50*p+qt]
                    # (pair tokens contiguous per j so the final projection's
                    # stationary AP has a single free dim)
                    if cT_both is None:
                        cT_both = sb.tile([128, 400], BF16, name="ctsb",
                                          tag="ctsb", bufs=3)
                    cv = cT_both[:, :].rearrange(
                        "p (j b s) -> p j b s", j=4, b=2)
                    nc.vector.tensor_mul(
                        cv[0:64, :, p, :],
                        ctxT[0:64, 0:200].rearrange("p (j s) -> p j s", j=4),
                        rexp2[0:64, 0:200].rearrange("p (j s) -> p j s", j=4))
                    nc.vector.tensor_mul(
                        cv[64:128, :, p, :],
                        ctxT[64:128, 0:200].rearrange("p (j s) -> p j s", j=4),
                        rexp2[64:128, 200:400].rearrange(
                            "p (j s) -> p j s", j=4))

                if stage < 7:
                    continue
                # residual tile (unpadded: 100 real token rows per pair)
                b0 = ch * NB + 2 * bp
                qres_sb = sb.tile([100, D], BF16, name="qres", tag="qres",
                                  bufs=3)
                nc.sync.dma_start(
                    out=qres_sb[:, :],
                    in_=qres_d[S * b0:S * b0 + 2 * S, :])

                # residual first (opens the PSUM group), then the pair-merged
                # final projection (lhsT free = 100 = both batches of the
                # pair via a [[200,2],[1,50]] AP) accumulates onto it
                po = ps.tile([128, D], FP32, name="po", tag="psD", bufs=2)
                nc.tensor.matmul(
                    po[0:2 * S, :], lhsT=ident[0:100, 0:2 * S],
                    rhs=qres_sb[:, :],
                    start=True, stop=False, skip_group_check=True,
                )
                for j in range(4):
                    nc.tensor.matmul(
                        po[0:2 * S, :],
                        lhsT=cT_both[:, 100 * j:100 * j + 2 * S],
                        rhs=wfc_sb[j][:, :],
                        start=False, stop=(j == 3), skip_group_check=True,
                    )

                if stage < 9:
                    continue
                # LayerNorm stats; sqrt is batched once per chunk (ACT table)
                stats = sb.tile([128, 6], FP32, name="stats", tag="stats",
                                bufs=4)
                nc.vector.bn_stats(out=stats[0:2 * S, :], in_=po[0:2 * S, :])
                nc.vector.bn_aggr(out=mvall[0:2 * S, 2 * bp:2 * bp + 2],
                                  in_=stats[0:2 * S, :])
                out1 = sb.tile([100, D], FP32, name="out1", tag="out1",
                               bufs=NB // 2 + 1)
                nc.vector.tensor_scalar(
                    out=out1[:, :], in0=po[0:2 * S, :],
                    scalar1=mvall[0:2 * S, 2 * bp:2 * bp + 1], scalar2=None,
                    op0=mybir.AluOpType.subtract,
                )
                out1_list.append(out1)

            if stage < 9:
                continue
            # batched rstd + final scale + store
            stdall = sb.tile([128, NB // 2], FP32, name="stdall", tag="stdall",
                             bufs=2)
            rstdall = sb.tile([128, NB // 2], FP32, name="rstdall",
                              tag="rstdall", bufs=2)
            nc.scalar.activation(
                out=stdall[0:2 * S, :],
                in_=mvall[:, :].rearrange(
                    "p (b two) -> p two b", two=2)[0:2 * S, 1, :],
                func=mybir.ActivationFunctionType.Sqrt,
                bias=eps_sb[0:2 * S, :])
            nc.vector.reciprocal(rstdall[0:2 * S, :], stdall[0:2 * S, :])
            for bp in range(NB // 2):
                out_sb = sb.tile([100, D], FP32, name="outsb", tag="outsb",
                                 bufs=3)
                nc.vector.tensor_scalar_mul(
                    out_sb[:, :], out1_list[bp][:, :],
                    rstdall[0:2 * S, bp:bp + 1])
                b0 = ch * NB + 2 * bp
                nc.sync.dma_start(
                    out=out_d[S * b0:S * b0 + 2 * S, :], in_=out_sb[:, :])

    nc.compile()
    return nc


_NC_CACHE = {}


def _get_nc(n_chunk, use_tw, use_mask):
    key = (n_chunk, use_tw, use_mask)
    if key not in _NC_CACHE:
        _NC_CACHE[key] = _build_nc(n_chunk, use_tw, use_mask)
    return _NC_CACHE[key]


def _prep_core_inputs(Qc, Kc, Vc, n_chunk, weights, tw_extra, mask_extra):
    """Host-side layout prep for one core's shard (nb batches)."""
    nb = Qc.shape[0]
    t = nb * S
    tp = nb * SPAD
    bf = ml_dtypes.bfloat16
    qt = np.ascontiguousarray(Qc.reshape(t, D).T.astype(bf))
    kt = np.ascontiguousarray(Kc.reshape(t, D).T.astype(bf))
    vp = np.zeros((nb, SPAD, D), dtype=bf)
    vp[:, :S] = Vc
    vt = np.ascontiguousarray(vp.reshape(tp, D).T)
    qres = np.ascontiguousarray(Qc.reshape(t, D).astype(bf))
    m = {"QT": qt, "KT": kt, "VT": vt, "QRES": qres}
    m.update(weights)
    m.update(tw_extra)
    m.update(mask_extra)
    return m


def kernel(Q, K, V, attn_mask, W_Q, W_K, W_V, W_fc, time_weight):
    Q = np.asarray(Q, dtype=np.float32)
    K = np.asarray(K, dtype=np.float32)
    V = np.asarray(V, dtype=np.float32)
    W_Q = np.asarray(W_Q, dtype=np.float32)
    W_K = np.asarray(W_K, dtype=np.float32)
    W_V = np.asarray(W_V, dtype=np.float32)
    W_fc = np.asarray(W_fc, dtype=np.float32)
    time_weight = np.asarray(time_weight, dtype=np.float32)
    attn_mask = np.asarray(attn_mask)

    bf = ml_dtypes.bfloat16
    use_tw = not np.all(time_weight == 1.0)
    use_mask = bool(np.any(attn_mask))

    weights = {
        "WQT": np.ascontiguousarray(W_Q.T.astype(bf)),
        "WKT": np.ascontiguousarray(W_K.T.astype(bf)),
        "WVT": np.ascontiguousarray(W_V.T.astype(bf)),
        "WFCT": np.ascontiguousarray(W_fc.T.astype(bf)),
    }
    tw_extra = {}
    if use_tw:
        # twT[kt, qt] for head h at rows 64*(h%2), cols 50*h
        twt = np.zeros((2 * SPAD, H * S), dtype=bf)
        twtt = time_weight.transpose(0, 2, 1)  # [h, kt, qt]
        for h in range(H):
            r0 = SPAD * (h % 2)
            c0 = 200 * (h % 2) + 50 * (h // 2)
            twt[r0:r0 + S, c0:c0 + S] = twtt[h].astype(bf)
        tw_extra["TWT"] = twt

    nc = _get_nc(NCHUNK, use_tw, use_mask)

    in_maps = []
    for c in range(NCORES):
        sl = slice(c * BC, (c + 1) * BC)
        mask_extra = {}
        if use_mask:
            mneg = np.zeros((BC, 2 * SPAD, H * S), dtype=np.float32)
            mc = attn_mask[sl]  # [BC, qt, kt]
            mt = np.where(mc, np.float32(NEG * 8.0), np.float32(0.0))
            # scoresT layout: [kt, qt]; exp scale=1/8 applied after add
            mtt = mt.transpose(0, 2, 1)  # [b, kt, qt]
            for b in range(BC):
                for h in range(H):
                    r0 = SPAD * (h % 2)
                    c0 = 200 * (h % 2) + 50 * (h // 2)
                    mneg[b, r0:r0 + S, c0:c0 + S] = mtt[b]
            mask_extra["MASKNEG"] = mneg
        in_maps.append(
            _prep_core_inputs(Q[sl], K[sl], V[sl], NCHUNK, weights,
                              tw_extra, mask_extra))

    res = run_bass_kernel_spmd(nc, in_maps, list(range(NCORES)),
                               trace=PROFILE, **PROFILE_KW)
    global LAST_RESULT
    LAST_RESULT = res
    out = np.empty((B, S, D), dtype=np.float32)
    for c in range(NCORES):
        out[c * BC:(c + 1) * BC] = res.results[c]["OUT"].reshape(BC, S, D)
    return out


PROFILE = False
PROFILE_KW = {}
LAST_RESULT = None
